# revision 1
# baseline (speedup 1.0000x reference)
"""CoAttLayer Trainium2 kernel.

Data-parallel over batch: 64 batches -> 8 NeuronCores x 8 batches.
Per batch (T = N = 1024, d = 64, k = 128):
    L  = tanh(R @ Wl @ P^T)                      (T, N)
    Hp = tanh(Wp @ P^T + (Wr @ R^T) @ L)         (k, N)
    Hr = tanh(Wr @ R^T + (Wp @ P^T) @ L^T)       (k, T)
    Ap = softmax(whp @ Hp), Ar = softmax(whr @ Hr)
    out = [P^T @ Ap ; R^T @ Ar]                  (2d,)

Layout strategy: all d-contractions run on partitions 0-63 (R^T, P^T, A^T and
the transposed small weights all live there).  L is produced tile-wise in PSUM
(t on partitions), tanh'd by ScalarE straight into fp16 SBUF, and L^T is
produced by the DMA xbar transpose (fp16) so neither the PE nor the DVE pays
for the big transpose.  Big matmuls run in float32r (full PE rate at free-dim
512); the L-sized operands run in fp16.
"""

import numpy as np
from contextlib import ExitStack

B, T, N, D, K = 64, 1024, 1024, 64, 128
NCORES = 8
BL = B // NCORES  # batches per core

_CACHE = {}


def _build():
    import concourse.tile as tile
    from concourse import bacc, mybir
    from concourse.masks import make_identity

    f32 = mybir.dt.float32
    f32r = mybir.dt.float32r
    f16 = mybir.dt.float16
    Tanh = mybir.ActivationFunctionType.Tanh
    Exp = mybir.ActivationFunctionType.Exp

    nc = bacc.Bacc(trn_type="TRN2")

    rv = nc.dram_tensor("review_seq", (BL, T, D), f32r, kind="ExternalInput")
    po = nc.dram_tensor("post_seq", (BL, N, D), f32r, kind="ExternalInput")
    wl = nc.dram_tensor("Wl", (D, D), f32r, kind="ExternalInput")
    wr = nc.dram_tensor("Wr", (K, D), f32r, kind="ExternalInput")
    wp = nc.dram_tensor("Wp", (K, D), f32r, kind="ExternalInput")
    whr = nc.dram_tensor("whr", (1, K), f32, kind="ExternalInput")
    whp = nc.dram_tensor("whp", (1, K), f32, kind="ExternalInput")
    out = nc.dram_tensor("out", (BL, 2 * D), f32, kind="ExternalOutput")
    import os
    DBG = bool(int(os.environ.get("KBDBG", "0")))
    if DBG:
        dbg_lf = nc.dram_tensor("dbg_lf", (BL, 128, 8, 1024), f16, kind="ExternalOutput")
        dbg_lt = nc.dram_tensor("dbg_lt", (BL, 128, 8, 1024), f16, kind="ExternalOutput")
        dbg_hp = nc.dram_tensor("dbg_hp", (BL, 128, 1024), f16, kind="ExternalOutput")
        dbg_hr = nc.dram_tensor("dbg_hr", (BL, 128, 1024), f16, kind="ExternalOutput")
        dbg_ee = nc.dram_tensor("dbg_ee", (BL, 128, 16), f16, kind="ExternalOutput")

    NT = T // 128  # 8 t-tiles
    NN = N // 128  # 8 n-tiles

    with tile.TileContext(nc) as tc, ExitStack() as ctx:
        singles = ctx.enter_context(tc.tile_pool(name="singles", bufs=1))
        sb = ctx.enter_context(tc.tile_pool(name="sb", bufs=2))
        pa = ctx.enter_context(tc.tile_pool(name="pa", bufs=2, space="PSUM"))
        pb = ctx.enter_context(tc.tile_pool(name="pb", bufs=2, space="PSUM"))

        # ---- per-core constants -------------------------------------------
        ident32 = singles.tile([128, 128], f32)
        make_identity(nc, ident32)
        ident = singles.tile([128, 128], f32r)
        nc.vector.tensor_copy(ident, ident32)
        one11 = singles.tile([1, 1], f32)
        nc.vector.memset(one11, 1.0)
        ident16 = singles.tile([128, 128], f16)
        nc.vector.tensor_copy(ident16, ident32)

        wl_sb = singles.tile([64, 64], f32r)
        nc.sync.dma_start(out=wl_sb, in_=wl[:, :])
        wl16 = singles.tile([64, 64], f16)
        nc.vector.tensor_copy(wl16, wl_sb)
        wr_sb = singles.tile([128, 64], f32r)
        nc.sync.dma_start(out=wr_sb, in_=wr[:, :])
        wp_sb = singles.tile([128, 64], f32r)
        nc.sync.dma_start(out=wp_sb, in_=wp[:, :])
        whp_sb = singles.tile([1, 128], f32)
        nc.sync.dma_start(out=whp_sb, in_=whp[:, :])
        whr_sb = singles.tile([1, 128], f32)
        nc.sync.dma_start(out=whr_sb, in_=whr[:, :])

        # Wr^T, Wp^T on partitions 0-63; whp^T/whr^T as fp16 columns.
        ps_w = pa.tile([128, 1024], f32r, tag="pa")
        nc.tensor.transpose(ps_w[0:64, 0:128], wr_sb, ident)
        nc.tensor.transpose(ps_w[0:64, 128:256], wp_sb, ident)
        wrT = singles.tile([64, 128], f16)
        nc.vector.tensor_copy(wrT, ps_w[0:64, 0:128])
        wpT = singles.tile([64, 128], f16)
        nc.vector.tensor_copy(wpT, ps_w[0:64, 128:256])
        ps_wh = pa.tile([128, 2], f32, tag="pa")
        nc.tensor.transpose(ps_wh[0:128, 0:1], whp_sb, one11)
        nc.tensor.transpose(ps_wh[0:128, 1:2], whr_sb, one11)
        whT = singles.tile([128, 2], f16)
        nc.vector.tensor_copy(whT, ps_wh)

        # ---- per-batch pipeline, software-pipelined emission ---------------
        # Sub-phases interleaved across three consecutive batches so each
        # engine's in-order stream has its dependencies ready just-in-time:
        #   A1: loads + input transposes + Rt/Pt evacuation
        #   A2: A^T, G_r, G_p matmuls + their fp16 casts
        #   A3: G transposes + evacuation, Pe/Re prep (gpsimd)
        #   B1: L tiles (matmul+tanh) with L^T transposes trailing one tile
        #   B2: Hp accumulation + tanh      B3: Hr accumulation + tanh
        #   C : logits, exp, pooling, output
        # Emission per iteration k: A1(k+2) B1(k+1) A2(k+2) B2(k+1) A3(k+2)
        # B3(k+1) C(k).
        st = {}

        def phaseA1(b):
            s = st[b] = {}
            s["RP"] = RP = sb.tile(name="rp", shape=[128, NT, 64], dtype=f32r, tag="rp", bufs=3)
            s["PP"] = PP = sb.tile(name="pp", shape=[128, NN, 64], dtype=f32r, tag="pp", bufs=3)
            nc.sync.dma_start(out=RP, in_=rv[b, :, :].rearrange("(i p) d -> p i d", p=128))
            nc.sync.dma_start(out=PP, in_=po[b, :, :].rearrange("(i p) d -> p i d", p=128))

            s["R16"] = R16 = sb.tile(name="r16", shape=[128, NT, 64], dtype=f16, tag="r16", bufs=3)
            nc.vector.tensor_copy(R16, RP)
            s["P16"] = P16 = sb.tile(name="p16", shape=[128, NN, 64], dtype=f16, tag="p16", bufs=3)
            nc.vector.tensor_copy(P16, PP)

            ps_rt = pa.tile([128, 1024], f16, tag="pa", name="ps_rt")
            for i in range(NT):
                nc.tensor.transpose(ps_rt[0:64, 128 * i:128 * (i + 1)], R16[:, i, :], ident16)
            s["Rt"] = Rt = sb.tile(name="rt", shape=[64, 1024], dtype=f16, tag="rt", bufs=3)
            nc.vector.tensor_copy(Rt, ps_rt[0:64, :])

            ps_pt = pa.tile([128, 1024], f16, tag="pa", name="ps_pt")
            for i in range(NN):
                nc.tensor.transpose(ps_pt[0:64, 128 * i:128 * (i + 1)], P16[:, i, :], ident16)
            s["Pt"] = Pt = sb.tile(name="pt", shape=[64, 1024], dtype=f16, tag="pt", bufs=3)
            nc.vector.tensor_copy(Pt, ps_pt[0:64, :])

        def phaseA2(b):
            s = st[b]
            Rt, Pt = s["Rt"], s["Pt"]
            ps_at = pa.tile([128, 1024], f32, tag="pa")
            nc.tensor.matmul(ps_at[0:64, 0:512], wl16, Rt[:, 0:512], start=True, stop=True)
            nc.tensor.matmul(ps_at[0:64, 512:1024], wl16, Rt[:, 512:1024], start=True, stop=True)
            s["AT"] = AT = sb.tile(name="at", shape=[64, 1024], dtype=f16, tag="at", bufs=3)
            nc.vector.tensor_copy(AT, ps_at[0:64, :])

        def phaseA3(b):
            s = st[b]
            RP, PP = s["RP"], s["PP"]
            Rt, Pt = s["Rt"], s["Pt"]
            # Gr^T = R @ Wr^T and Gp^T = P @ Wp^T computed directly
            s["GT"] = GT = sb.tile(name="gt", shape=[128, NT + NN, 128], dtype=f16, tag="gt", bufs=3)
            ps_gtr = pb.tile([128, NT, 128], f32, tag="pb")
            for a in range(NT):
                nc.tensor.matmul(ps_gtr[:, a, :], Rt[:, 128 * a:128 * (a + 1)], wrT,
                                 start=True, stop=True)
            nc.vector.tensor_copy(GT[:, 0:NT, :], ps_gtr)
            ps_gtp = pb.tile([128, NN, 128], f32, tag="pb")
            for a in range(NN):
                nc.tensor.matmul(ps_gtp[:, a, :], Pt[:, 128 * a:128 * (a + 1)], wpT,
                                 start=True, stop=True)
            nc.vector.tensor_copy(GT[:, NT:NT + NN, :], ps_gtp)

            # pooling rhs with ones column (gpsimd: off the DVE)
            s["Pe"] = Pe = sb.tile(name="pe", shape=[128, NN, 65], dtype=f16, tag="pe", bufs=3)
            nc.gpsimd.tensor_copy(out=Pe[:, :, 0:64], in_=PP)
            nc.gpsimd.memset(Pe[:, :, 64:65], 1.0)
            s["Re"] = Re = sb.tile(name="re", shape=[128, NT, 65], dtype=f16, tag="re", bufs=3)
            nc.gpsimd.tensor_copy(out=Re[:, :, 0:64], in_=RP)
            nc.gpsimd.memset(Re[:, :, 64:65], 1.0)

        def phaseB1(b):
            s = st[b]
            Pt, AT = s["Pt"], s["AT"]
            # L tiles: L_i = tanh(A_i @ P^T) -> fp16 ; L^T via PE transposes
            s["Lf"] = Lf = sb.tile(name="lf", shape=[128, NT, 1024], dtype=f16, tag="lf")
            s["LT"] = LT = sb.tile(name="lt", shape=[128, NN, 1024], dtype=f16, tag="lt")

            def l_transposes(i):
                ps_lt = pa.tile([128, NN, 128], f16, tag="pa")
                for j in range(NN):
                    nc.tensor.transpose(ps_lt[:, j, :], Lf[:, i, 128 * j:128 * (j + 1)], ident16)
                nc.vector.tensor_copy(LT[:, :, 128 * i:128 * (i + 1)], ps_lt)

            for i in range(NT):
                ps_l = pa.tile([128, 1024], f32, tag="pa")
                lhs = AT[:, 128 * i:128 * (i + 1)]
                nc.tensor.matmul(ps_l[:, 0:512], lhs, Pt[:, 0:512], start=True, stop=True)
                nc.tensor.matmul(ps_l[:, 512:1024], lhs, Pt[:, 512:1024], start=True, stop=True)
                nc.scalar.activation(Lf[:, i, :], ps_l, Tanh)
                if i > 0:
                    l_transposes(i - 1)
            l_transposes(NT - 1)

        def phaseB2(b):
            s = st[b]
            Pt, GT, Lf = s["Pt"], s["GT"], s["Lf"]
            # Hp = tanh(G_p + sum_t G_r^T.T @ L)   (k, n)
            ps_hp = pb.tile([128, 1024], f32, tag="pb")
            nc.tensor.matmul(ps_hp[:, 0:512], wpT, Pt[:, 0:512], start=True, stop=False)
            nc.tensor.matmul(ps_hp[:, 512:1024], wpT, Pt[:, 512:1024], start=True, stop=False)
            for j in range(NT):
                nc.tensor.matmul(ps_hp[:, 0:512], GT[:, j, :], Lf[:, j, 0:512],
                                 start=False, stop=(j == NT - 1))
                nc.tensor.matmul(ps_hp[:, 512:1024], GT[:, j, :], Lf[:, j, 512:1024],
                                 start=False, stop=(j == NT - 1))
            s["Hp16"] = Hp16 = sb.tile(name="hp16", shape=[128, 1024], dtype=f16, tag="hp16")
            nc.scalar.activation(Hp16, ps_hp, Tanh)

        def phaseB3(b):
            s = st[b]
            Rt, GT, LT = s["Rt"], s["GT"], s["LT"]
            # Hr = tanh(G_r + sum_n G_p^T.T @ L^T)   (k, t)
            ps_hr = pb.tile([128, 1024], f32, tag="pb")
            nc.tensor.matmul(ps_hr[:, 0:512], wrT, Rt[:, 0:512], start=True, stop=False)
            nc.tensor.matmul(ps_hr[:, 512:1024], wrT, Rt[:, 512:1024], start=True, stop=False)
            for j in range(NN):
                nc.tensor.matmul(ps_hr[:, 0:512], GT[:, NT + j, :], LT[:, j, 0:512],
                                 start=False, stop=(j == NN - 1))
                nc.tensor.matmul(ps_hr[:, 512:1024], GT[:, NT + j, :], LT[:, j, 512:1024],
                                 start=False, stop=(j == NN - 1))
            s["Hr16"] = Hr16 = sb.tile(name="hr16", shape=[128, 1024], dtype=f16, tag="hr16")
            nc.scalar.activation(Hr16, ps_hr, Tanh)

        def phaseC(b):
            s = st.pop(b)
            Hp16, Hr16 = s["Hp16"], s["Hr16"]
            Pe, Re = s["Pe"], s["Re"]
            Lf, LT = s["Lf"], s["LT"]
            # logits^T: (n,1) and (t,1) per 128-chunk, then exp (no max-sub:
            # |logit| <= ||wh||_1 ~ 5, exp stays in fp16 range)
            ps_lg = pa.tile([128, 16], f32, tag="pa")
            for i in range(NN):
                nc.tensor.matmul(ps_lg[:, i:i + 1], Hp16[:, 128 * i:128 * (i + 1)],
                                 whT[:, 0:1], start=True, stop=True)
            for i in range(NT):
                nc.tensor.matmul(ps_lg[:, 8 + i:9 + i], Hr16[:, 128 * i:128 * (i + 1)],
                                 whT[:, 1:2], start=True, stop=True)
            ee = sb.tile([128, 16], f16, tag="ee")
            nc.scalar.activation(ee, ps_lg, Exp)

            ps_co = pa.tile([128, 1024], f32, tag="pa")
            for j in range(NN):
                nc.tensor.matmul(ps_co[0:1, 0:65], ee[:, j:j + 1], Pe[:, j, :],
                                 start=(j == 0), stop=(j == NN - 1))
            for j in range(NT):
                nc.tensor.matmul(ps_co[0:1, 512:577], ee[:, 8 + j:9 + j], Re[:, j, :],
                                 start=(j == 0), stop=(j == NT - 1))

            if DBG:
                nc.sync.dma_start(out=dbg_lf[b], in_=Lf)
                nc.sync.dma_start(out=dbg_lt[b], in_=LT)
                nc.sync.dma_start(out=dbg_hp[b], in_=Hp16)
                nc.sync.dma_start(out=dbg_hr[b], in_=Hr16)
                nc.sync.dma_start(out=dbg_ee[b], in_=ee)
            rinv = sb.tile([1, 2], f32, tag="rinv")
            nc.vector.reciprocal(rinv[0:1, 0:1], ps_co[0:1, 64:65])
            nc.vector.reciprocal(rinv[0:1, 1:2], ps_co[0:1, 576:577])
            ob = sb.tile([1, 128], f32, tag="ob")
            nc.vector.tensor_scalar_mul(ob[0:1, 0:64], ps_co[0:1, 0:64], rinv[0:1, 0:1])
            nc.vector.tensor_scalar_mul(ob[0:1, 64:128], ps_co[0:1, 512:576], rinv[0:1, 1:2])
            nc.sync.dma_start(out=out[b:b + 1, :], in_=ob)

        def fullA(b):
            phaseA1(b); phaseA2(b); phaseA3(b)

        fullA(0)
        if BL > 1:
            phaseA1(1)
            phaseB1(0)
            phaseA2(1)
            phaseB2(0)
            phaseA3(1)
            phaseB3(0)
        else:
            phaseB1(0); phaseB2(0); phaseB3(0)
        for k in range(BL):
            if k + 2 < BL:
                phaseA1(k + 2)
            if k + 1 < BL:
                phaseB1(k + 1)
            if k + 2 < BL:
                phaseA2(k + 2)
            if k + 1 < BL:
                phaseB2(k + 1)
            if k + 2 < BL:
                phaseA3(k + 2)
            if k + 1 < BL:
                phaseB3(k + 1)
            phaseC(k)

    nc.compile()
    return nc


def get_nc():
    if "nc" not in _CACHE:
        _CACHE["nc"] = _build()
    return _CACHE["nc"]


def make_in_maps(inputs):
    R = np.ascontiguousarray(inputs["review_seq"], dtype=np.float32)
    P = np.ascontiguousarray(inputs["post_seq"], dtype=np.float32)
    w = {
        "Wl": np.ascontiguousarray(inputs["Wl"], dtype=np.float32),
        "Wr": np.ascontiguousarray(inputs["Wr"], dtype=np.float32),
        "Wp": np.ascontiguousarray(inputs["Wp"], dtype=np.float32),
        "whr": np.ascontiguousarray(inputs["whr"], dtype=np.float32),
        "whp": np.ascontiguousarray(inputs["whp"], dtype=np.float32),
    }
    in_maps = []
    for c in range(NCORES):
        m = {
            "review_seq": np.ascontiguousarray(R[c * BL:(c + 1) * BL]),
            "post_seq": np.ascontiguousarray(P[c * BL:(c + 1) * BL]),
        }
        m.update(w)
        in_maps.append(m)
    return in_maps


def run(inputs, trace=False):
    from concourse.bass_utils import run_bass_kernel_spmd

    nc = get_nc()
    res = run_bass_kernel_spmd(nc, make_in_maps(inputs),
                               core_ids=list(range(NCORES)), trace=trace)
    outp = np.concatenate([r["out"] for r in res.results], axis=0)
    return outp.astype(np.float32), res


def kernel(**inputs) -> np.ndarray:
    outp, _ = run(inputs, trace=False)
    return outp



# revision 16
# speedup vs baseline: 1.1865x; 1.1865x over previous
"""CoAttLayer Trainium2 kernel.

Data-parallel over batch: 64 batches -> 8 NeuronCores x 8 batches.
Per batch (T = N = 1024, d = 64, k = 128):
    L  = tanh(R @ Wl @ P^T)                      (T, N)
    Hp = tanh(Wp @ P^T + (Wr @ R^T) @ L)         (k, N)
    Hr = tanh(Wr @ R^T + (Wp @ P^T) @ L^T)       (k, T)
    Ap = softmax(whp @ Hp), Ar = softmax(whr @ Hr)
    out = [P^T @ Ap ; R^T @ Ar]                  (2d,)

Layout strategy:
  * d-contractions (transposed R/P/A and small weights) live on partitions
    0-63; L tiles come out of PSUM t-major, tanh'd by ScalarE straight into
    fp8e4 SBUF.
  * The Hp and Hr accumulations over t/n (the two big 128-contraction
    passes) run as fp8e4 DoubleRow matmuls: two 128-row k-tiles per
    instruction at 0.5 cycles per moving column (4x the fp16 rate).
  * L^T is produced by the DMA xbar transpose operating on the fp8 L tiles
    viewed as fp16 byte-pairs.  Partition c of the transposed tile then
    holds the interleaved pair (n=2m, n=2m+1), m = 128g + c, which is
    exactly a DoubleRow k-tile pair.  The matching weights (Gp^T) are
    computed directly in that parity-packed layout with stride-2 lhsT
    slices of P^T, so no shuffle is ever needed.  This takes the big
    transpose off both the PE and the DVE.
"""

import numpy as np
from contextlib import ExitStack

B, T, N, D, K = 64, 1024, 1024, 64, 128
NCORES = 8
BL = B // NCORES  # batches per core

_CACHE = {}


def _build():
    import concourse.tile as tile
    from concourse import bacc, mybir
    from concourse.masks import make_identity

    f32 = mybir.dt.float32
    f32r = mybir.dt.float32r
    f16 = mybir.dt.float16
    f8 = mybir.dt.float8e4
    DR = mybir.MatmulPerfMode.DoubleRow
    Tanh = mybir.ActivationFunctionType.Tanh
    Exp = mybir.ActivationFunctionType.Exp

    nc = bacc.Bacc(trn_type="TRN2")

    rv = nc.dram_tensor("review_seq", (BL, T, D), f32r, kind="ExternalInput")
    po = nc.dram_tensor("post_seq", (BL, N, D), f32r, kind="ExternalInput")
    wl = nc.dram_tensor("Wl", (D, D), f32r, kind="ExternalInput")
    wr = nc.dram_tensor("Wr", (K, D), f32r, kind="ExternalInput")
    wp = nc.dram_tensor("Wp", (K, D), f32r, kind="ExternalInput")
    whr = nc.dram_tensor("whr", (1, K), f32, kind="ExternalInput")
    whp = nc.dram_tensor("whp", (1, K), f32, kind="ExternalInput")
    out = nc.dram_tensor("out", (BL, 2 * D), f32, kind="ExternalOutput")

    NT = T // 128  # 8 t-tiles
    NN = N // 128  # 8 n-tiles
    NG = N // 256  # 4 DoubleRow pair-chunks

    with tile.TileContext(nc) as tc, ExitStack() as ctx:
        singles = ctx.enter_context(tc.tile_pool(name="singles", bufs=1))
        sb = ctx.enter_context(tc.tile_pool(name="sb", bufs=2))
        # pa: 4 x 1-bank buffers (all pa tiles are <=2KB/partition) so the
        # ps_l rotation is 4 deep and the PE never waits on ScalarE tanh.
        pa = ctx.enter_context(tc.tile_pool(name="pa", bufs=4, space="PSUM"))
        pb = ctx.enter_context(tc.tile_pool(name="pb", bufs=2, space="PSUM"))

        # ---- per-core constants -------------------------------------------
        ident32 = singles.tile([128, 128], f32)
        make_identity(nc, ident32)
        ident = singles.tile([128, 128], f32r)
        nc.vector.tensor_copy(ident, ident32)
        one11 = singles.tile([1, 1], f32)
        nc.vector.memset(one11, 1.0)
        ident16 = singles.tile([128, 128], f16)
        nc.vector.tensor_copy(ident16, ident32)

        wl_sb = singles.tile([64, 64], f32r)
        nc.sync.dma_start(out=wl_sb, in_=wl[:, :])
        wl16 = singles.tile([64, 64], f16)
        nc.vector.tensor_copy(wl16, wl_sb)
        wr_sb = singles.tile([128, 64], f32r)
        nc.sync.dma_start(out=wr_sb, in_=wr[:, :])
        wp_sb = singles.tile([128, 64], f32r)
        nc.sync.dma_start(out=wp_sb, in_=wp[:, :])
        whp_sb = singles.tile([1, 128], f32)
        nc.sync.dma_start(out=whp_sb, in_=whp[:, :])
        whr_sb = singles.tile([1, 128], f32)
        nc.sync.dma_start(out=whr_sb, in_=whr[:, :])

        # Wr^T, Wp^T on partitions 0-63; whp^T/whr^T as fp16 columns.
        ps_w = pb.tile([128, 1024], f32r, tag="pb")
        nc.tensor.transpose(ps_w[0:64, 0:128], wr_sb, ident)
        nc.tensor.transpose(ps_w[0:64, 128:256], wp_sb, ident)
        wrT = singles.tile([64, 128], f16)
        nc.vector.tensor_copy(wrT, ps_w[0:64, 0:128])
        wpT = singles.tile([64, 128], f16)
        nc.vector.tensor_copy(wpT, ps_w[0:64, 128:256])
        ps_wh = pb.tile([128, 2], f32, tag="pb")
        nc.tensor.transpose(ps_wh[0:128, 0:1], whp_sb, one11)
        nc.tensor.transpose(ps_wh[0:128, 1:2], whr_sb, one11)
        whT = singles.tile([128, 2], f16)
        nc.vector.tensor_copy(whT, ps_wh)

        # ---- per-batch pipeline, software-pipelined emission ---------------
        # Emission per iteration k: A1(k+2) B1(k+1) A2(k+2) B2(k+1) A3(k+2)
        # B3(k+1) C(k).  The L^T DMA transposes issued inside B1(k+1) are
        # covered by the A2/B2/A3 emissions before B3(k+1) consumes them.
        st = {}

        def phaseA1(b):
            s = st[b] = {}
            s["RP"] = RP = sb.tile(name="rp", shape=[128, NT, 64], dtype=f32r, tag="rp", bufs=3)
            s["PP"] = PP = sb.tile(name="pp", shape=[128, NN, 64], dtype=f32r, tag="pp", bufs=3)
            # input loads ride the Activation hwdge queue; sharing the SP
            # queue with the L^T transposes corrupts the transpose
            # completion ordering (seen as co_r errors on early batches).
            nc.scalar.dma_start(out=RP, in_=rv[b, :, :].rearrange("(i p) d -> p i d", p=128))
            nc.scalar.dma_start(out=PP, in_=po[b, :, :].rearrange("(i p) d -> p i d", p=128))

            s["R16"] = R16 = sb.tile(name="r16", shape=[128, NT, 64], dtype=f16, tag="r16", bufs=3)
            nc.vector.tensor_copy(R16, RP)
            s["P16"] = P16 = sb.tile(name="p16", shape=[128, NN, 64], dtype=f16, tag="p16", bufs=3)
            nc.vector.tensor_copy(P16, PP)

            ps_rt = pa.tile([128, 1024], f16, tag="pa", name="ps_rt")
            for i in range(NT):
                nc.tensor.transpose(ps_rt[0:64, 128 * i:128 * (i + 1)], R16[:, i, :], ident16)
            s["Rt"] = Rt = sb.tile(name="rt", shape=[64, 1024], dtype=f16, tag="rt", bufs=3)
            nc.vector.tensor_copy(Rt, ps_rt[0:64, :])

            ps_pt = pa.tile([128, 1024], f16, tag="pa", name="ps_pt")
            for i in range(NN):
                nc.tensor.transpose(ps_pt[0:64, 128 * i:128 * (i + 1)], P16[:, i, :], ident16)
            s["Pt"] = Pt = sb.tile(name="pt", shape=[64, 1024], dtype=f16, tag="pt", bufs=3)
            nc.vector.tensor_copy(Pt, ps_pt[0:64, :])

        def phaseA2(b):
            s = st[b]
            Rt = s["Rt"]
            s["AT"] = AT = sb.tile(name="at", shape=[64, 1024], dtype=f16, tag="at", bufs=3)
            for h in range(2):
                ps_at = pa.tile([64, 512], f32, tag="pa")
                nc.tensor.matmul(ps_at, wl16, Rt[:, 512 * h:512 * (h + 1)],
                                 start=True, stop=True)
                nc.vector.tensor_copy(AT[:, 512 * h:512 * (h + 1)], ps_at)

        def phaseA3(b):
            s = st[b]
            RP, PP = s["RP"], s["PP"]
            Rt, Pt = s["Rt"], s["Pt"]
            # Gr^T tiles (Hp DoubleRow weights): Gr^T = R @ Wr^T, t-major.
            s["GTr"] = GTr = sb.tile(name="gtr", shape=[128, NT, 128], dtype=f8, tag="gtr", bufs=3)
            ps_gtr = pb.tile([128, NT, 128], f32, tag="pb")
            for a in range(NT):
                nc.tensor.matmul(ps_gtr[:, a, :], Rt[:, 128 * a:128 * (a + 1)], wrT,
                                 start=True, stop=True)
            nc.vector.tensor_copy(GTr, ps_gtr)

            # Gp^T in parity-packed layout (Hr DoubleRow weights):
            # GTp[c, g, i, k] = Gp^T[256 g + 2 c + i, k], via stride-2 slices
            # of P^T as the stationary operand.
            s["GTp"] = GTp = sb.tile(name="gtp", shape=[128, NG, 2, 128], dtype=f8, tag="gtp", bufs=3)
            PtI = Pt[:, :].rearrange("d (m i) -> d i m", i=2)
            ps_gtp = pb.tile([128, NG, 2, 128], f32, tag="pb")
            for g in range(NG):
                for par in range(2):
                    nc.tensor.matmul(ps_gtp[:, g, par, :],
                                     PtI[:, par, 128 * g:128 * (g + 1)], wpT,
                                     start=True, stop=True)
            nc.vector.tensor_copy(GTp, ps_gtp)

            # fused pooling rhs [P | 1 | R] (gpsimd: off the DVE); the single
            # ones column yields both softmax denominators (row 0 = sum of
            # ee_p, row 1 = sum of ee_r).
            s["PRe"] = PRe = sb.tile(name="pre", shape=[128, NN, 129], dtype=f16, tag="pre", bufs=3)
            nc.gpsimd.tensor_copy(out=PRe[:, :, 0:64], in_=PP)
            nc.gpsimd.memset(PRe[:, :, 64:65], 1.0)
            nc.gpsimd.tensor_copy(out=PRe[:, :, 65:129], in_=RP)

        def phaseB1(b):
            s = st[b]
            Pt, AT = s["Pt"], s["AT"]
            # L tiles: L_i = tanh(A_i @ P^T) -> fp8 straight from ScalarE.
            # L^T via DMA xbar transpose of the fp8 pairs viewed as fp16:
            # LT16[c, g, t] <-> fp8 pair (n = 2(128g+c), n+1) at column t.
            s["Lf"] = Lf = sb.tile(name="lf", shape=[128, NT, 1024], dtype=f8, tag="lf", bufs=3)
            s["LT16"] = LT16 = sb.tile(name="lt16", shape=[128, NG, 1024], dtype=f16, tag="lt16", bufs=3)

            for i in range(NT):
                lhs = AT[:, 128 * i:128 * (i + 1)]
                for h in range(2):
                    ps_l = pa.tile([128, 512], f32, tag="pa")
                    nc.tensor.matmul(ps_l, lhs, Pt[:, 512 * h:512 * (h + 1)],
                                     start=True, stop=True)
                    nc.scalar.activation(Lf[:, i, 512 * h:512 * (h + 1)], ps_l, Tanh)
                nc.sync.dma_start_transpose(
                    out=LT16[:, :, 128 * i:128 * (i + 1)],
                    in_=Lf[:, i, :].bitcast(f16))

        def phaseB2(b):
            s = st[b]
            Pt, GTr, Lf = s["Pt"], s["GTr"], s["Lf"]
            # Hp = tanh(Wp @ P^T + sum_t Gr^T.T @ L), DoubleRow over t-tile
            # pairs.
            ps_hp = pb.tile([128, 1024], f32, tag="pb")
            nc.tensor.matmul(ps_hp[:, 0:512], wpT, Pt[:, 0:512], start=True, stop=False)
            nc.tensor.matmul(ps_hp[:, 512:1024], wpT, Pt[:, 512:1024], start=True, stop=False)
            for a in range(NT // 2):
                last = a == NT // 2 - 1
                nc.tensor.matmul(ps_hp[:, 0:512], GTr[:, 2 * a:2 * a + 2, :],
                                 Lf[:, 2 * a:2 * a + 2, 0:512],
                                 start=False, stop=last, perf_mode=DR)
                nc.tensor.matmul(ps_hp[:, 512:1024], GTr[:, 2 * a:2 * a + 2, :],
                                 Lf[:, 2 * a:2 * a + 2, 512:1024],
                                 start=False, stop=last, perf_mode=DR)
            s["Hp16"] = Hp16 = sb.tile(name="hp16", shape=[128, 1024], dtype=f16, tag="hp16")
            nc.scalar.activation(Hp16, ps_hp, Tanh)

        def phaseB3(b):
            s = st[b]
            Rt, GTp, LT16 = s["Rt"], s["GTp"], s["LT16"]
            # Hr = tanh(Wr @ R^T + sum_n Gp^T.T @ L^T), DoubleRow over the
            # parity-packed pairs produced by the DMA transpose.
            ps_hr = pb.tile([128, 1024], f32, tag="pb")
            nc.tensor.matmul(ps_hr[:, 0:512], wrT, Rt[:, 0:512], start=True, stop=False)
            nc.tensor.matmul(ps_hr[:, 512:1024], wrT, Rt[:, 512:1024], start=True, stop=False)
            for g in range(NG):
                last = g == NG - 1
                for h in range(2):
                    rhs = LT16[:, g, 512 * h:512 * (h + 1)].bitcast(f8) \
                        .rearrange("c (t i) -> c i t", i=2)
                    nc.tensor.matmul(ps_hr[:, 512 * h:512 * (h + 1)],
                                     GTp[:, g, :, :], rhs,
                                     start=False, stop=last, perf_mode=DR)
            s["Hr16"] = Hr16 = sb.tile(name="hr16", shape=[128, 1024], dtype=f16, tag="hr16")
            nc.scalar.activation(Hr16, ps_hr, Tanh)

        def phaseC(b):
            s = st.pop(b)
            Hp16, Hr16 = s["Hp16"], s["Hr16"]
            PRe = s["PRe"]
            # logits^T: (n,1) and (t,1) per 128-chunk, then exp (no max-sub:
            # |logit| <= ||wh||_1 ~ 5, exp stays in fp16 range)
            ps_lg = pb.tile([128, 16], f32, tag="pb")
            for i in range(NN):
                nc.tensor.matmul(ps_lg[:, i:i + 1], Hp16[:, 128 * i:128 * (i + 1)],
                                 whT[:, 0:1], start=True, stop=True)
            for i in range(NT):
                nc.tensor.matmul(ps_lg[:, 8 + i:9 + i], Hr16[:, 128 * i:128 * (i + 1)],
                                 whT[:, 1:2], start=True, stop=True)
            ee = sb.tile([128, 16], f16, tag="ee")
            nc.scalar.activation(ee, ps_lg, Exp)

            # fused pooling: lhsT = (ee_p_j, ee_r_j), rhs = [P|1|R|1] ->
            # row 0 carries the P-side sums, row 1 the R-side (off-diagonal
            # quadrants are unused).
            eeR = ee[:, :].rearrange("p (s j) -> p j s", s=2)
            ps_co = pb.tile([2, 129], f32, tag="pb")
            for j in range(NN):
                nc.tensor.matmul(ps_co, eeR[:, j, :], PRe[:, j, :],
                                 start=(j == 0), stop=(j == NN - 1))

            rinv = sb.tile([2, 1], f32, tag="rinv")
            nc.vector.reciprocal(rinv, ps_co[0:2, 64:65])
            ob = sb.tile([2, 129], f32, tag="ob")
            nc.vector.tensor_scalar_mul(ob, ps_co[0:2, :], rinv)
            nc.sync.dma_start(out=out[b:b + 1, 0:64], in_=ob[0:1, 0:64])
            nc.sync.dma_start(out=out[b:b + 1, 64:128], in_=ob[1:2, 65:129])

        # B1 (and its L^T DMA transposes) runs a full iteration ahead of
        # B3, so each transpose has ~1.5 iterations of cover before its
        # consumer.  C(k) is emitted before B2/B3(k+1) so the pb PSUM
        # rotation never waits on a just-issued tanh.
        phaseA1(0); phaseA2(0); phaseB1(0); phaseA3(0)
        if BL > 1:
            phaseA1(1); phaseA2(1); phaseB1(1); phaseA3(1)
            phaseB2(0)
            phaseB3(0)
        else:
            phaseB2(0); phaseB3(0)
        for k in range(BL):
            if k + 2 < BL:
                phaseA1(k + 2)
                phaseA2(k + 2)
                phaseB1(k + 2)
                phaseA3(k + 2)
            phaseC(k)
            if k + 1 < BL:
                phaseB2(k + 1)
                phaseB3(k + 1)

    nc.compile()
    return nc


def get_nc():
    if "nc" not in _CACHE:
        _CACHE["nc"] = _build()
    return _CACHE["nc"]


def make_in_maps(inputs):
    R = np.ascontiguousarray(inputs["review_seq"], dtype=np.float32)
    P = np.ascontiguousarray(inputs["post_seq"], dtype=np.float32)
    w = {
        "Wl": np.ascontiguousarray(inputs["Wl"], dtype=np.float32),
        "Wr": np.ascontiguousarray(inputs["Wr"], dtype=np.float32),
        "Wp": np.ascontiguousarray(inputs["Wp"], dtype=np.float32),
        "whr": np.ascontiguousarray(inputs["whr"], dtype=np.float32),
        "whp": np.ascontiguousarray(inputs["whp"], dtype=np.float32),
    }
    in_maps = []
    for c in range(NCORES):
        m = {
            "review_seq": np.ascontiguousarray(R[c * BL:(c + 1) * BL]),
            "post_seq": np.ascontiguousarray(P[c * BL:(c + 1) * BL]),
        }
        m.update(w)
        in_maps.append(m)
    return in_maps


def run(inputs, trace=False):
    from concourse.bass_utils import run_bass_kernel_spmd

    nc = get_nc()
    res = run_bass_kernel_spmd(nc, make_in_maps(inputs),
                               core_ids=list(range(NCORES)), trace=trace)
    outp = np.concatenate([r["out"] for r in res.results], axis=0)
    return outp.astype(np.float32), res


def kernel(**inputs) -> np.ndarray:
    outp, _ = run(inputs, trace=False)
    return outp
